# revision 14
# baseline (speedup 1.0000x reference)
"""Trainium2 Bass kernel: batched multi-head attention, data-parallel over batch.

Full inputs: query/key/value (8, 12, 512, 256) fp32; 8 heads x head_dim 32;
softmax over the 512 keys for each (batch, step, head, query-node).

Sharding: batch (=8) across the 8 NeuronCores; each core computes its
(12, 512, 256) slice independently. No collectives.

Per-core algorithm, per step t (12 steps), per head-half (4 heads):
  - Load Q,K natural as bf16 (cast DMA), xbar-DMA-transpose 128x128 blocks to
    get Q^T,K^T with head_dim on partitions.
  - scores_T[k,q] = K^T-chunk.T @ Q^T: contraction is head_dim=32, so two
    heads run concurrently in the PE array via row tiling (tile_position).
  - exp of the scores is split across TWO engines per half-step: ScalarE
    computes exact exp on 5 of the 8 (128,1024) score tiles; VectorE computes
    the other 3 via a custom DVE op implementing the Schraudolph bit trick:
    int16(x*C0 + C1) produces the bit pattern of bf16 2^(x*SCALE*log2e) with
    ~+-3% per-element error that averages out in the softmax (verified
    end-to-end rel err ~1e-2 < 2e-2 budget). This splits the former single
    ScalarE exp bottleneck (~220us) across ACT and DVE (~137us each).
  - out_T = [V | ones].T @ exp_T accumulated over the 4 k-chunks; the ones
    column yields the softmax denominator for free. Two heads per PSUM bank
    (partitions 0-32 / 64-96, concurrently via column tiling). V is loaded
    pre-interleaved [V_h | 1] by a strided cast-DMA (no DVE interleave pass).
  - One fused DVE copy moves the (97,1024) AV PSUM block to SBUF; PE
    transposes it back to (q, d); reciprocal + broadcast-multiply read the
    transpose PSUM directly (no SBUF compaction pass); DMA out.
  - The (QK+exp) stage of each half is emitted one half-stage AHEAD of the
    AV+output stage so the PE always has the next half's scores ready for
    ACT/DVE (keeps both exp engines gap-free at t boundaries).
"""

import numpy as np

import concourse.bass as bass
import concourse.mybir as mybir
import concourse.tile as tile
from concourse import bacc
from concourse.masks import make_identity
from concourse.bass_utils import run_bass_kernel_spmd

B, T, N, D = 8, 12, 512, 256
H, HD = 8, 32
SCALE = 1.0 / float(np.sqrt(HD))
NCORES = 8

F32 = mybir.dt.float32
BF16 = mybir.dt.bfloat16

QK_DTYPE = BF16   # Q/K operand dtype (bf16 enables xbar DMA transpose)
EXP_DTYPE = BF16  # sbuf dtype of exp(scores) == AV rhs operand dtype
V_DTYPE = BF16    # sbuf dtype of V (+ones)  == AV lhsT operand dtype
import os as _os0
PIPE_STAGES = int(_os0.environ.get("K_PIPE", "2"))  # half-stage lookahead

# ---- custom DVE op: Schraudolph exp -> bf16 bit pattern --------------------
# out_bits(int16) = round(x*C0 + C1); viewed as bf16 this is
# 2^(x*SCALE*log2e + centering) = exp(x*SCALE) * (1 + eps), |eps| <~ 3%.
# C0 maps raw scores to 128ths-of-exponent units; C1 = (127 - c)*128 with
# c = E_f[log2(1+f) - f] = 0.057305 centering the tent-shaped log error.
SCHR_C0 = 128.0 * float(np.log2(np.e)) * SCALE
SCHR_C1 = (127.0 - 0.057304959111036) * 128.0
# score tiles (kc, pr) routed to the DVE each half-step; rest go to ScalarE.
# Priority order spreads approximated tiles across key-chunks first.
_DVE_ORDER = [(0, 1), (1, 1), (2, 1), (0, 0), (1, 0), (2, 0), (3, 1), (3, 0)]
import os as _os
N_DVE = int(_os.environ.get("K_N_DVE", "4"))   # tiles/half routed to DVE
AVS_ENGINE = _os.environ.get("K_AVS", "act")   # 'act' | 'dve' | 'alt'
DVE_TILES = frozenset(_DVE_ORDER[:N_DVE])


def _register_schraudolph_op():
    import concourse.dve_ops as dve_ops
    from concourse.dve_spec import Spec, Src0, C0, C1, lower, _has_src1
    from concourse.dve_uop import DveOpSpec

    name = "SCHRAUDOLPH_EXP_ANT"
    for o in dve_ops.OPS:
        if o.name == name:
            return o
    spec = Spec(
        body=Src0 * C0 + C1,
        reference=lambda in0, in1, s0, s1, imm2: (
            in0.astype(np.float32) * np.float32(s0) + np.float32(s1)
        ),
    )
    row = dve_ops._CUSTOM_DVE_ROW_BASE + len(dve_ops.OPS)
    dve_ops._SUB_OPCODE_FOR_NAME[name] = row
    shas = {}
    for ver in ("v3", "v4"):
        s = DveOpSpec(
            name=name, opcode=row, uops=lower(spec, ver=ver),
            rd1_en=_has_src1(spec),
        )
        shas[ver] = s.sha(ver)
    op = dve_ops.DveOp(name, spec, subdim=False, uops_sha=shas)
    dve_ops.OPS.append(op)
    dve_ops.CUSTOM_DVE_SPECS[name] = spec
    return op


SCHR_OP = _register_schraudolph_op()


def _attention_body(tc, o_ext, q_ext, k_ext, v_ext, reps=1):
    nc = tc.nc
    Exp = mybir.ActivationFunctionType.Exp

    with (
        tc.tile_pool(name="const", bufs=1) as const_pool,
        tc.tile_pool(name="qk", bufs=3) as qk_pool,
        tc.tile_pool(name="vsb", bufs=4) as v_pool,
        tc.tile_pool(name="exp", bufs=24) as exp_pool,
        tc.tile_pool(name="avs", bufs=3) as avs_pool,
        tc.tile_pool(name="rec", bufs=3) as rec_pool,
        tc.tile_pool(name="fout", bufs=4) as fout_pool,
        tc.tile_pool(name="scorep", bufs=2, space="PSUM") as scores_pool,
        tc.tile_pool(name="avp", bufs=1, space="PSUM") as av_pool,
        tc.tile_pool(name="trp", bufs=1, space="PSUM") as tr_pool,
    ):
        ident = const_pool.tile([128, 128], F32)
        warm = const_pool.tile([128, 1], F32)
        vset = {}
        for s in range(2):
            for half in range(2):
                vsb = v_pool.tile([128, 4, 132], V_DTYPE, tag="vsb")
                vset[s, half] = vsb
        qkT = {}

        def emit_preamble():
            # everything here is off the first-exp critical path; emitted
            # after t=0's Q/K loads so the DMA queues start those first
            make_identity(nc, ident[:])
            nc.scalar.activation(warm[:], ident[:, 0:1], Exp)  # exp table load
            # persistent V (+ones) tiles, double-buffered by t parity; the
            # ones columns are written once here and never overwritten.
            for s in range(2):
                for half in range(2):
                    nc.gpsimd.memset(vset[s, half][:], 1.0)

        def emit_qk_loads(j, t):
            # natural-layout loads (cast f32 -> bf16 during DMA, SWDGE),
            # split per head-half so half-0 transposes can start while the
            # half-1 columns are still loading
            qnat = qk_pool.tile([128, 4, 256], QK_DTYPE, tag="qnat")
            knat = qk_pool.tile([128, 4, 256], QK_DTYPE, tag="knat")
            for half in range(2):
                cols = slice(half * 128, (half + 1) * 128)
                nc.gpsimd.dma_start(
                    out=qnat[:, :, cols],
                    in_=q_ext[t].rearrange("(c p) d -> p c d", p=128)[
                        :, :, cols
                    ],
                )
                nc.gpsimd.dma_start(
                    out=knat[:, :, cols],
                    in_=k_ext[t].rearrange("(c p) d -> p c d", p=128)[
                        :, :, cols
                    ],
                )

            # transposes via xbar DMA: qT[d, half, q] (head_dim on partitions)
            qT = qk_pool.tile([128, 2, 512], QK_DTYPE, tag="qT")
            kT = qk_pool.tile([128, 2, 512], QK_DTYPE, tag="kT")
            for half in range(2):
                for c in range(4):
                    nc.sync.dma_start_transpose(
                        out=qT[:, half, c * 128:(c + 1) * 128],
                        in_=qnat[:, c, half * 128:(half + 1) * 128],
                    )
                    nc.sync.dma_start_transpose(
                        out=kT[:, half, c * 128:(c + 1) * 128],
                        in_=knat[:, c, half * 128:(half + 1) * 128],
                    )
            qkT[j] = (qT, kT)

        def emit_v_load(t):
            # V: one natural cast-load (f32 -> bf16 DMA), then a cheap bf16
            # DVE interleave into the persistent [V_h | 1] tiles (the ones
            # columns at offset 32 of each 33-group survive the memset)
            vtmp = qk_pool.tile([128, 4, 256], V_DTYPE, tag="vtmp")
            nc.gpsimd.dma_start(
                out=vtmp[:], in_=v_ext[t].rearrange("(kc p) d -> p kc d", p=128)
            )
            for half in range(2):
                nc.vector.tensor_copy(
                    vset[t % 2, half][:].rearrange(
                        "p kc (h x) -> p kc h x", x=33
                    )[:, :, :, 0:32],
                    vtmp[:, :, half * 128:(half + 1) * 128].rearrange(
                        "p kc (h d) -> p kc h d", d=32
                    ),
                )

        def emit_qk_exp(j, t, half):
            qT, kT = qkT[j]
            exps = {}
            for kc in range(4):
                for pr in range(2):
                    scores = scores_pool.tile([128, 1024], F32)  # 2 banks
                    for sub in range(2):
                        r = pr * 64 + sub * 32
                        nc.tensor.matmul(
                            scores[:, sub * 512:(sub + 1) * 512],
                            lhsT=kT[r:r + 32, half, kc * 128:(kc + 1) * 128],
                            rhs=qT[r:r + 32, half, :],
                            start=True, stop=True,
                            tile_position=(r, 0),
                        )
                    exp_t = exp_pool.tile([128, 1024], EXP_DTYPE, tag="exp")
                    if (kc, pr) in DVE_TILES:
                        nc.vector._custom_dve(
                            SCHR_OP,
                            out=exp_t[:].bitcast(mybir.dt.int16),
                            in0=scores[:],
                            s0=SCHR_C0, s1=SCHR_C1,
                        )
                    else:
                        nc.scalar.activation(
                            exp_t[:], scores[:], Exp, scale=SCALE
                        )
                    exps[kc, pr] = exp_t
            return exps

        def emit_av_out(t, half, exps):
            # AV with ones-column denominators; all 4 heads of the half in
            # one 2-bank PSUM tile so the SBUF spill is a single DVE copy
            av = av_pool.tile([128, 1024], F32)
            for b in range(2):
                for j in range(2):
                    i = b * 2 + j
                    for kc in range(4):
                        nc.tensor.matmul(
                            av[64 * j:64 * j + 33, b * 512:(b + 1) * 512],
                            lhsT=vset[t % 2, half][:, kc, i * 33:(i + 1) * 33],
                            rhs=exps[kc, i // 2][
                                :, (i % 2) * 512:(i % 2 + 1) * 512
                            ],
                            start=(kc == 0), stop=(kc == 3),
                        )

            # PSUM -> SBUF spill: on ScalarE it lands right after the next
            # half's exp ops in the ACT queue, so the single-buffered av tile
            # frees exactly when the next AV group needs it (no PE<->DVE
            # ping-pong); 'alt' splits the cost across both engines.
            avs = avs_pool.tile([128, 1024], F32, tag="avs")
            eng = AVS_ENGINE
            if eng == "alt":
                eng = "act" if (t * 2 + half) % 2 == 0 else "dve"
            if eng == "act":
                nc.scalar.copy(avs[0:97, :], av[0:97, :])
            else:
                nc.vector.tensor_copy(avs[0:97, :], av[0:97, :])

            # transpose back to (q, d) orientation, 128x128 blocks on the PE
            trp = tr_pool.tile([128, 1024], F32)
            for b in range(2):
                for c in range(4):
                    nc.tensor.transpose(
                        trp[:, b * 512 + c * 128:b * 512 + (c + 1) * 128],
                        avs[:, b * 512 + c * 128:b * 512 + (c + 1) * 128],
                        ident[:],
                    )

            # denominators sit at offset 32 of each 64-col group of trp;
            # reciprocal + broadcast-multiply read the transpose PSUM direct
            rec = rec_pool.tile([128, 16], F32, tag="rec")
            nc.vector.reciprocal(
                rec[:].rearrange("p (x e) -> p x e", e=1),
                trp[:].rearrange("p (x e) -> p x e", e=64)[:, :, 32:33],
            )

            fout = fout_pool.tile([128, 512], F32, tag="fout")
            for b in range(2):
                in0 = trp[:, b * 512:(b + 1) * 512].rearrange(
                    "p (c j e) -> p c j e", j=2, e=64
                )[:, :, :, 0:32]
                outap = fout[:].rearrange(
                    "p (c x d) -> p c x d", x=4, d=32
                )[:, :, b * 2:b * 2 + 2, :]
                recap = rec[:].rearrange(
                    "p (bb c j) -> p bb c j", bb=2, c=4
                )[:, b].unsqueeze(3).broadcast_to((128, 4, 2, 32))
                nc.vector.tensor_mul(outap, in0, recap)

            nc.sync.dma_start(
                out=o_ext[t].rearrange("(c p) d -> p c d", p=128)[
                    :, :, half * 128:(half + 1) * 128
                ],
                in_=fout[:].rearrange("p (c d) -> p c d", d=128),
            )

        # Q/K loads+transposes are emitted one t-step AHEAD so the xbar DMA
        # latency never stalls the first QK matmul of a step.
        steps = [t for _ in range(reps) for t in range(T)]
        pending = []
        first = True
        for j in range(len(steps)):
            t = steps[j]
            for half in range(2):
                if half == 0:
                    if first:
                        emit_qk_loads(j, t)
                        emit_preamble()
                        first = False
                    if j + 1 < len(steps):
                        emit_qk_loads(j + 1, steps[j + 1])
                    emit_v_load(t)
                pending.append((t, half, emit_qk_exp(j, t, half)))
                if len(pending) > PIPE_STAGES:
                    emit_av_out(*pending.pop(0))
        for item in pending:
            emit_av_out(*item)


def build_program(enable_asserts=False, reps=1):
    nc = bacc.Bacc(
        "TRN2",
        target_bir_lowering=False,
        debug=False,
        enable_asserts=enable_asserts,
        num_devices=NCORES,
    )
    q_ext = nc.dram_tensor("q", [T, N, D], F32, kind="ExternalInput").ap()
    k_ext = nc.dram_tensor("k", [T, N, D], F32, kind="ExternalInput").ap()
    v_ext = nc.dram_tensor("v", [T, N, D], F32, kind="ExternalInput").ap()
    o_ext = nc.dram_tensor("out", [T, N, D], F32, kind="ExternalOutput").ap()
    with tile.TileContext(nc) as tc:
        _attention_body(tc, o_ext, q_ext, k_ext, v_ext, reps=reps)
    nc.compile()
    return nc


_NC_CACHE = None


def _get_nc():
    global _NC_CACHE
    if _NC_CACHE is None:
        _NC_CACHE = build_program()
    return _NC_CACHE


def run(query, key, value, trace=False):
    """Run on the 8 NeuronCores; returns (out, exec_time_ns_or_None)."""
    nc = _get_nc()
    in_maps = [
        {
            "q": np.ascontiguousarray(np.asarray(query[i], dtype=np.float32)),
            "k": np.ascontiguousarray(np.asarray(key[i], dtype=np.float32)),
            "v": np.ascontiguousarray(np.asarray(value[i], dtype=np.float32)),
        }
        for i in range(NCORES)
    ]
    res = run_bass_kernel_spmd(nc, in_maps, core_ids=list(range(NCORES)),
                               trace=trace)
    out = np.stack([np.asarray(res.results[i]["out"]) for i in range(NCORES)])
    return out, res.exec_time_ns


def kernel(query, key, value):
    out, _ = run(query, key, value, trace=False)
    return out
